# revision 2
# baseline (speedup 1.0000x reference)
"""DyBEM layer (histogram binning + embedding sum + linear) on 8 trn2 cores.

Math reduction (same as baseline):
  out[b] = IN_DIM*(EW[0] + b) + sum_k g[b,k] * (EW[k+1] - EW[k]),
  g[b,k] = #{n : x[b,n] > T[k,n]},  T[k,n] = gmin[n] + bins[k]*(grange[n]+eps)

v2 schedule (cc mode):
  t~0-3.6   x loads (SP c0,c1 | Pool c2,c3); params on Act; PE transposes
            (f32r, 1.5cyc/row) chase chunks; DVE min / Pool max accumulate
  t~5.4     lstat -> cc_in DMA -> AllGather (Pool, 15.1us cost-model)
  cc window param PE chain (EW, D10), dblk built via fp16 DRAM bounce +
            mask matmul, base bias, PE ramp dummies gated on the collective
  cc end    post2 gather DMAs -> thresholds -> 9 indicators x 4 quarters
            (DVE k<6, Pool k>=6) -> 36+ matmuls quarter-major -> evict+store
            (last quarter split into eighths to shorten the tail)
"""

import numpy as np

import concourse.bass as bass
import concourse.mybir as mybir
import concourse.tile as tile
from concourse import bacc, bass_utils
from concourse.masks import make_identity

F32 = mybir.dt.float32
F32R = mybir.dt.float32r
F16 = mybir.dt.float16
ALU = mybir.AluOpType
AX = mybir.AxisListType
ACT = mybir.ActivationFunctionType

B_FULL, IN_DIM, NUM_BINS, EMBED_DIM = 32768, 64, 10, 64
N_CORES = 8
B_C = B_FULL // N_CORES          # 4096 rows per core
EPS = 1e-6
P = 128
T_ALL = B_C // P                 # 32 row-groups (t index)
NTHR = NUM_BINS - 1              # 9 real thresholds
UCOLS = B_C // 2                 # 2048 u columns (2 rows per column)
MM_N = 512                       # matmul moving free size (one PSUM bank)
N_CHUNKS = 4                     # x load chunks of 512 f32
CSZ = T_ALL * IN_DIM // N_CHUNKS  # 512

EXCHANGE = "cc"                  # "rdma" | "cc"
N_DUMMY = 8                      # PE ramp warmers gated on the collective


def _patch_nc_mappings():
    """fake_nrt can't answer libndbg NC/routing queries; fall back to the
    identity mapping (any bijection keeps the XOR exchange correct)."""
    from concourse import libnrt

    orig_nc = libnrt.get_trn2_nc_mapping
    orig_rid = libnrt.get_device_id_to_routing_id_mapping

    def nc_safe():
        try:
            return orig_nc()
        except Exception:
            return {(d, i): i for d in range(16) for i in range(8)}

    def rid_safe():
        try:
            return orig_rid()
        except Exception:
            return {d: d for d in range(16)}

    libnrt.get_trn2_nc_mapping = nc_safe
    libnrt.get_device_id_to_routing_id_mapping = rid_safe
    try:
        import concourse.bass_interp as bi

        bi.get_device_id_to_routing_id_mapping = rid_safe
    except Exception:
        pass


_patch_nc_mappings()


def _trace_kernel(tc, io, it=0, deferred_waits=None, tag=""):
    nc = tc.nc
    x_d, bl_d, emb_d, w_d, b_d, out_d = io

    with (
        tc.tile_pool(name=f"const{tag}", bufs=1) as cpool,
        tc.tile_pool(name=f"ind{tag}", bufs=9) as ipool,
        tc.tile_pool(name=f"outs{tag}", bufs=4) as opool,
        tc.tile_pool(name=f"tp_psum{tag}", bufs=2, space="PSUM") as tp_psum,
        tc.tile_pool(name=f"out_psum{tag}", bufs=4, space="PSUM") as out_psum,
        tc.tile_pool(name=f"mc_psum{tag}", bufs=2, space="PSUM") as mc_psum,
        tc.tile_pool(name=f"dram{tag}", bufs=1, space="DRAM") as dpool,
    ):
        # ---- x loads first: SP c0,c1 | Pool c2,c3 (scalar kept free: its
        # queue starts with the Exp act-table load) ----
        x_nat = cpool.tile([P, T_ALL * IN_DIM], F32R)
        x_view = x_d.rearrange("(p t) n -> p (t n)", p=P).bitcast(F32R)
        lqs = (nc.sync, nc.sync, nc.gpsimd, nc.gpsimd)
        for c in range(N_CHUNKS):
            sl = slice(c * CSZ, (c + 1) * CSZ)
            lqs[c].dma_start(x_nat[:, sl], x_view[:, sl])

        ident = cpool.tile([P, P], F32)
        make_identity(nc, ident[:])
        ident_r = cpool.tile([P, P], F32R)
        nc.vector.tensor_copy(ident_r[:], ident[:])
        identr = ident_r[:]

        # param input DMAs (scalar queue; needed only inside the cc window)
        bl_row = cpool.tile([1, NUM_BINS], F32)
        nc.scalar.dma_start(bl_row[:], bl_d.unsqueeze(0))
        emb_s = cpool.tile([NUM_BINS, EMBED_DIM], F32)
        nc.scalar.dma_start(emb_s[:], emb_d)
        w_s = cpool.tile([EMBED_DIM, EMBED_DIM], F32)
        nc.scalar.dma_start(w_s[:], w_d)

        # bins = cumsum(softmax(bin_logits))
        e_row = cpool.tile([1, NUM_BINS], F32)
        nc.scalar.activation(e_row[:], bl_row[:], ACT.Exp)
        ssum = cpool.tile([1, 1], F32)
        nc.vector.tensor_reduce(ssum[:], e_row[:], AX.X, ALU.add)
        rsum = cpool.tile([1, 1], F32)
        nc.vector.reciprocal(rsum[:], ssum[:])
        prob_a = cpool.tile([1, NUM_BINS], F32)
        nc.gpsimd.tensor_scalar(prob_a[:], e_row[:], rsum[:, 0:1], None, ALU.mult)
        prob_b = cpool.tile([1, NUM_BINS], F32)
        cur, nxt = prob_a, prob_b
        for sh in (1, 2, 4, 8):
            nc.gpsimd.tensor_copy(nxt[:, 0:sh], cur[:, 0:sh])
            nc.gpsimd.tensor_tensor(
                nxt[:, sh:NUM_BINS], cur[:, sh:NUM_BINS], cur[:, 0 : NUM_BINS - sh],
                ALU.add,
            )
            cur, nxt = nxt, cur
        bins_d = dpool.tile([1, NUM_BINS], F32)
        nc.scalar.dma_start(bins_d[:], cur[:])
        bins_bc = cpool.tile([P, NUM_BINS], F32)
        nc.scalar.dma_start(bins_bc[:], bins_d[:].broadcast_to([P, NUM_BINS]))

        # ---- per-chunk: transposes (PE) + min/max accumulate (DVE-only:
        # gpsimd ucode has no TensorTensor min/max; it seeds via copies) ----
        macc_min = cpool.tile([P, CSZ], F32)
        macc_max = cpool.tile([P, CSZ], F32)
        u_t = cpool.tile([P, UCOLS], F32)
        for c in range(N_CHUNKS):
            sl = slice(c * CSZ, (c + 1) * CSZ)
            xf = x_nat[:, sl].bitcast(F32)
            if c == 0:
                nc.gpsimd.tensor_copy(macc_min[:], xf)
                nc.gpsimd.tensor_copy(macc_max[:], xf)
            else:
                nc.vector.tensor_tensor(macc_min[:], macc_min[:], xf, ALU.min)
                nc.vector.tensor_tensor(macc_max[:], macc_max[:], xf, ALU.max)
            ps_tp = tp_psum.tile([P, 4 * P], F32R, tag="tp")
            for jj in range(4):
                j = c * 4 + jj
                nc.tensor.transpose(
                    ps_tp[:, jj * P : (jj + 1) * P],
                    x_nat[:, j * P : (j + 1) * P],
                    identr,
                )
            nc.scalar.activation(
                u_t[:, c * 4 * P : (c + 1) * 4 * P], ps_tp[:].bitcast(F32),
                ACT.Copy,
            )

        # ---- stat folds -> lstat [128,1] = (min[n] | -(max[n]+eps)) ----
        stat128 = cpool.tile([P, P], F32)
        nc.vector.tensor_reduce(
            stat128[:, 0:64],
            macc_min[:].rearrange("p (t n) -> p n t", t=CSZ // IN_DIM),
            AX.X, ALU.min,
        )
        nc.vector.tensor_reduce(
            stat128[:, 64:P],
            macc_max[:].rearrange("p (t n) -> p n t", t=CSZ // IN_DIM),
            AX.X, ALU.max,
        )
        ps_st = mc_psum.tile([P, P], F32, tag="mc")
        nc.tensor.transpose(ps_st[:], stat128[:], ident[:])
        lstat = cpool.tile([P, 1], F32)
        nc.vector.tensor_reduce(lstat[0:64, :], ps_st[0:64, :], AX.X, ALU.min)
        nc.vector.tensor_reduce(lstat[64:P, :], ps_st[64:P, :], AX.X, ALU.max)
        nc.vector.tensor_scalar(
            lstat[64:P, :], lstat[64:P, :], -1.0, -EPS, ALU.mult, ALU.add
        )

        # ---- cross-core exchange ----
        cc_instr = None
        if EXCHANGE == "rdma":
            pmin = cpool.tile([P, P], F32R)
            pmax = cpool.tile([P, P], F32R)
            for msk, base0 in ((pmin, 0), (pmax, -64)):
                nc.vector.memset(msk[:], 0.0)
                for half in range(2):
                    hsl = slice(half * 64, (half + 1) * 64)
                    nc.gpsimd.affine_select(
                        out=msk[:, hsl], in_=msk[:, hsl],
                        compare_op=ALU.not_equal, fill=1.0, base=base0,
                        pattern=[[-1, 64]], channel_multiplier=1,
                    )
            ones_col = cpool.tile([P, 1], F32R)
            nc.vector.memset(ones_col[:], 1.0)
            lsem = nc.alloc_semaphore(f"rl{tag}") if it == 0 else _SEMS["l"]
            dsem = nc.alloc_semaphore(f"rd{tag}") if it == 0 else _SEMS["d"]
            rsems = (
                [nc.alloc_semaphore(f"rs{s}{tag}") for s in range(3)]
                if it == 0
                else _SEMS["r"]
            )
            if it == 0:
                _SEMS.update({"l": lsem, "d": dsem, "r": rsems})
            accs = [lstat] + [
                cpool.tile([P, 1], F32R, name=f"acc{s}") for s in range(1, 4)
            ]
            rbufs = [cpool.tile([P, 1], F32R, name=f"rbuf{s}") for s in range(3)]
            for s in range(3):
                d = 1 << s
                rdests = [None] * 8
                rdests[4 if d == 4 else 0] = (0, d)
                nc.gpsimd.remote_dma_broadcast(
                    rbufs[s][:], accs[s][:], rsems[s], lsem, rdests=rdests
                )
                trig = nc.gpsimd.trigger_dma(count=None)
                k = 3 * it + s + 1
                trig.then_inc(dsem, 1)
                w = nc.vector.wait_ge(dsem, k)
                comb = nc.vector.tensor_tensor(
                    accs[s + 1][:], accs[s][:], rbufs[s][:], ALU.min
                )
                tile.add_dep_helper(
                    comb.ins, w.ins, sync=True, reason="rdma arrival order"
                )
                deferred_waits.append((dsem.num, k, rsems[s], 2 * (it + 1)))
            gstat = accs[3]
            lpmin = cpool.tile([P, P], F32R)
            lpmax = cpool.tile([P, P], F32R)
            nc.vector.tensor_scalar(lpmin[:], pmin[:], gstat[:, 0:1], None, ALU.mult)
            nc.gpsimd.tensor_scalar(lpmax[:], pmax[:], gstat[:, 0:1], None, ALU.mult)
            ps_pr = mc_psum.tile([P, 2], F32, tag="mc")
            nc.tensor.matmul(ps_pr[:, 0:1], lpmin[:], ones_col[:],
                             start=True, stop=True)
            nc.tensor.matmul(ps_pr[:, 1:2], lpmax[:], ones_col[:],
                             start=True, stop=True)
            pr = cpool.tile([P, 2], F32)
            nc.vector.tensor_copy(pr[:], ps_pr[:])
        else:
            cc_in = dpool.tile([1, P], F32)
            nc.sync.dma_start(cc_in[:], lstat[:])
            cc_out = dpool.tile([N_CORES, P], F32, addr_space="Shared")
            cc_instr = nc.gpsimd.collective_compute(
                "AllGather",
                ALU.bypass,
                replica_groups=[list(range(N_CORES))],
                ins=[cc_in[:]],
                outs=[cc_out[:]],
            )

        # ---- work hidden inside the cc window: EW / D10 / dblk / base ----
        ps_embT = mc_psum.tile([EMBED_DIM, NUM_BINS], F32, tag="mc")
        nc.tensor.transpose(ps_embT[:], emb_s[:], ident[0:NUM_BINS, 0:NUM_BINS])
        embT_s = cpool.tile([EMBED_DIM, NUM_BINS], F32)
        nc.scalar.activation(embT_s[:], ps_embT[:], ACT.Copy)

        ps_wt = mc_psum.tile([EMBED_DIM, EMBED_DIM], F32, tag="mc")
        nc.tensor.transpose(ps_wt[:], w_s[:], ident[0:EMBED_DIM, 0:EMBED_DIM])
        wt_s = cpool.tile([EMBED_DIM, EMBED_DIM], F32)
        nc.scalar.activation(wt_s[:], ps_wt[:], ACT.Copy)

        ps_ew = mc_psum.tile([NUM_BINS, EMBED_DIM], F32, tag="mc")
        nc.tensor.matmul(ps_ew[:], embT_s[:], wt_s[:], start=True, stop=True)
        ew_ext = cpool.tile([NUM_BINS + 1, EMBED_DIM], F32)
        nc.scalar.activation(ew_ext[0:NUM_BINS, :], ps_ew[:], ACT.Copy)
        nc.sync.dma_start(
            ew_ext[NUM_BINS : NUM_BINS + 1, :], b_d.unsqueeze(0)
        )

        mt = cpool.tile([NUM_BINS + 1, NUM_BINS], F32)
        nc.gpsimd.memset(mt[:], 0.0)
        nc.gpsimd.affine_select(
            out=mt[:, 0:NTHR], in_=mt[:, 0:NTHR], compare_op=ALU.not_equal,
            fill=-1.0, base=0, pattern=[[-1, NTHR]], channel_multiplier=1,
        )
        nc.gpsimd.affine_select(
            out=mt[:, 0:NTHR], in_=mt[:, 0:NTHR], compare_op=ALU.not_equal,
            fill=1.0, base=-1, pattern=[[-1, NTHR]], channel_multiplier=1,
        )
        nc.gpsimd.affine_select(
            out=mt[:, NTHR : NTHR + 1], in_=mt[:, NTHR : NTHR + 1],
            compare_op=ALU.not_equal, fill=float(IN_DIM),
            base=0, pattern=[[-1, 1]], channel_multiplier=1,
        )
        nc.gpsimd.affine_select(
            out=mt[:, NTHR : NTHR + 1], in_=mt[:, NTHR : NTHR + 1],
            compare_op=ALU.not_equal, fill=float(IN_DIM),
            base=-NUM_BINS, pattern=[[-1, 1]], channel_multiplier=1,
        )
        ps_d10 = mc_psum.tile([NUM_BINS, EMBED_DIM], F32, tag="mc")
        nc.tensor.matmul(ps_d10[:], mt[:], ew_ext[:], start=True, stop=True)
        d10_h = cpool.tile([NUM_BINS, EMBED_DIM], F16)
        nc.scalar.activation(d10_h[:], ps_d10[:], ACT.Copy)

        # dblk via fp16 DRAM bounce + mask matmul
        d10_d = dpool.tile([NUM_BINS, EMBED_DIM], F16)
        nc.sync.dma_start(d10_d[:], d10_h[:])
        mini = cpool.tile([2, NTHR * P], F16)
        nc.vector.memset(mini[:], 0.0)
        for h in range(2):
            mrow = mini[h : h + 1, :].rearrange("o (k g f) -> o k g f", k=NTHR, g=2)
            (nc.sync if h == 0 else nc.scalar).dma_start(
                mrow[:, :, h, :], d10_d[0:NTHR, :].unsqueeze(0)
            )
        amask = cpool.tile([2, P], F16)
        nc.vector.memset(amask[:], 1.0)
        nc.gpsimd.affine_select(
            out=amask[:], in_=amask[:], compare_op=ALU.is_ge,
            fill=0.0, base=0, pattern=[[1, P]], channel_multiplier=-64,
        )
        nc.gpsimd.affine_select(
            out=amask[:], in_=amask[:], compare_op=ALU.is_ge,
            fill=0.0, base=63, pattern=[[-1, P]], channel_multiplier=64,
        )
        dblk = cpool.tile([P, NTHR * P], F32R)
        for j in range(3):
            dsl = slice(j * 384, (j + 1) * 384)
            ps_db = mc_psum.tile([P, 384], F32, tag="mc", name=f"psdb{j}")
            nc.tensor.matmul(ps_db[:], amask[:], mini[:, dsl], start=True, stop=True)
            nc.vector.tensor_copy(dblk[:, dsl], ps_db[:])

        # base bias: D10 row 9 partition-broadcast via the fp16 bounce
        base16 = cpool.tile([P, 1], F16)
        brow = d10_d[NTHR : NTHR + 1, :].squeeze(0).unsqueeze(1)  # [64, 1]
        for h in range(2):
            (nc.sync if h == 0 else nc.scalar).dma_start(
                base16[h * 64 : (h + 1) * 64, :], brow
            )
        base_col = cpool.tile([P, 1], F32)
        nc.vector.tensor_copy(base_col[:], base16[:])

        # PE ramp dummies: gated on the collective so they run at its tail
        dummy_mms = []
        if cc_instr is not None and N_DUMMY:
            for dmy in range(N_DUMMY):
                ps_dm = tp_psum.tile([P, P], F32R, tag="tp", name=f"dmy{dmy}")
                mm = nc.tensor.transpose(ps_dm[:], identr, identr)
                tile.add_dep_helper(mm.ins, cc_instr.ins, sync=True,
                                    reason="pe ramp warmup at cc tail")
                dummy_mms.append(mm)

        # ---- post-cc: gather stats + thresholds ----
        if EXCHANGE == "cc":
            # post2[(par,n), (s,r)] = cc_out[r, s*64+n]; 4 small strided DMAs
            # spread over 3 queues
            post2 = cpool.tile([P, 2 * N_CORES], F32)
            pq = (nc.sync, nc.scalar, nc.gpsimd, nc.scalar)
            post2_dmas = []
            for s in range(2):
                for h in range(2):
                    src = cc_out[:, s * 64 : (s + 1) * 64].transpose([1, 0])
                    post2_dmas.append(pq[s * 2 + h].dma_start(
                        post2[h * 64 : (h + 1) * 64,
                              s * N_CORES : (s + 1) * N_CORES],
                        src,
                    ))
            for mm in dummy_mms[-3:]:
                tile.add_dep_helper(mm.ins, post2_dmas[0].ins, sync=True,
                                    reason="pe ramp warmup to first matmul")
            pr = cpool.tile([P, 2], F32)
            nc.vector.tensor_reduce(
                pr[:], post2[:].rearrange("p (s r) -> p s r", s=2), AX.X, ALU.min
            )

        range_dup = cpool.tile([P, 1], F32)
        nc.vector.tensor_scalar(
            range_dup[:], pr[:, 1:2], -1.0, pr[:, 0:1], ALU.mult, ALU.subtract
        )
        s_thr = cpool.tile([P, NUM_BINS], F32)
        nc.vector.tensor_scalar(
            s_thr[:], bins_bc[:], range_dup[:, 0:1], pr[:, 0:1], ALU.mult, ALU.add
        )

        # ---- indicators + matmuls + store, quarter-major; last quarter as
        # two eighths to shorten the tail ----
        blocks = [(0, MM_N), (MM_N, MM_N), (2 * MM_N, MM_N),
                  (3 * MM_N, MM_N // 2), (3 * MM_N + MM_N // 2, MM_N // 2)]
        inds = [
            ipool.tile([P, UCOLS], F32R, tag="ind", name=f"ind_{k}")
            for k in range(NTHR)
        ]
        for bi_, (off, width) in enumerate(blocks):
            qsl = slice(off, off + width)
            ps_o = out_psum.tile([P, width], F32, tag="out", name=f"pso_{bi_}")
            for k in range(NTHR):
                eng = nc.gpsimd if k >= 5 else nc.vector
                eng.tensor_scalar(
                    inds[k][:, qsl], u_t[:, qsl], s_thr[:, k : k + 1], None,
                    ALU.is_gt,
                )
                nc.tensor.matmul(
                    ps_o[:],
                    dblk[:, k * P : (k + 1) * P],
                    inds[k][:, qsl],
                    start=(k == 0),
                    stop=(k == NTHR - 1),
                )
            out_s = opool.tile([P, width], F32, tag="outs", name=f"outs_{bi_}")
            if bi_ == len(blocks) - 1:
                # last block: bias-add on DVE so it overlaps ACT's previous
                # eviction, shortening the tail
                nc.vector.tensor_scalar(
                    out_s[:], ps_o[:], base_col[:, 0:1], None, ALU.add
                )
            else:
                nc.scalar.activation(
                    out_s[:], ps_o[:], ACT.Identity, bias=base_col[:, 0:1]
                )
            nc.sync.dma_start(out_d[:, qsl], out_s[:])


_CACHED = {}
_SEMS = {}


def _build(loop=1):
    if loop in _CACHED:
        return _CACHED[loop]
    _SEMS.clear()
    nc = bacc.Bacc(
        "TRN2",
        target_bir_lowering=False,
        debug=False,
        enable_asserts=True,
        num_devices=N_CORES,
    )
    deferred = []
    with tile.TileContext(nc) as tc:
        io = (
            nc.dram_tensor("x_sh", [B_C, IN_DIM], F32, kind="ExternalInput").ap(),
            nc.dram_tensor("bin_logits", [NUM_BINS], F32, kind="ExternalInput").ap(),
            nc.dram_tensor("embed", [NUM_BINS, EMBED_DIM], F32, kind="ExternalInput").ap(),
            nc.dram_tensor("W", [EMBED_DIM, EMBED_DIM], F32, kind="ExternalInput").ap(),
            nc.dram_tensor("b", [EMBED_DIM], F32, kind="ExternalInput").ap(),
            nc.dram_tensor("out_t", [P, UCOLS], F32, kind="ExternalOutput").ap(),
        )
        for it in range(loop):
            _trace_kernel(tc, io, it, deferred, tag=f"_{it}" if loop > 1 else "")
    # Rewrite placeholder waits (dummy local sem) to the real remote sems —
    # the single-core scheduling pass can't satisfy remote-sem waits.
    patch = {(d, k): (sem.num, sem.name, val) for d, k, sem, val in deferred}
    n_patched = 0
    for fn in nc.m.functions:
        for bb in fn.blocks:
            for inst in bb.instructions:
                si = inst.sync_info
                if si is None:
                    continue
                for sw in si.on_wait:
                    key = (sw.id, sw.wait_value)
                    if sw.sync_type == "semaphore" and key in patch:
                        sid, sname, val = patch[key]
                        sw.id = sid
                        sw.ant_name = sname
                        sw.wait_value = val
                        n_patched += 1
    assert n_patched == len(patch), (n_patched, len(patch))
    nc.compile()
    _CACHED[loop] = nc
    return nc


def _make_in_maps(x, bin_logits, embed, W, b):
    maps = []
    for c in range(N_CORES):
        maps.append(
            {
                "x_sh": np.ascontiguousarray(x[c * B_C : (c + 1) * B_C]),
                "bin_logits": np.asarray(bin_logits),
                "embed": np.asarray(embed),
                "W": np.asarray(W),
                "b": np.asarray(b),
            }
        )
    return maps


def _unshard(results):
    shards = []
    for c in range(N_CORES):
        out_t = results[c]["out_t"]  # [128=(par,f), 2048=(j,p)]
        shard = (
            out_t.reshape(2, EMBED_DIM, T_ALL // 2, P)
            .transpose(3, 2, 0, 1)           # [p, j, par, f]
            .reshape(B_C, EMBED_DIM)         # b = p*32 + j*2 + par
        )
        shards.append(shard)
    return np.ascontiguousarray(np.concatenate(shards, axis=0))


def kernel(x, bin_logits, embed, W, b):
    nc = _build()
    in_maps = _make_in_maps(np.asarray(x, dtype=np.float32), bin_logits, embed, W, b)
    res = bass_utils.run_bass_kernel_spmd(nc, in_maps, core_ids=list(range(N_CORES)))
    return _unshard(res.results)
